# revision 16
# baseline (speedup 1.0000x reference)
"""Trainium2 Bass kernel for nn_ArcGenerator (dense transformer arc attention).

Sharding: data-parallel over batch B=8 -> one batch element per NeuronCore.
Everything is batch-local; no collectives. Weights replicated (host
pre-transposes them so every matmul contracts along SBUF partitions).

Per-core pipeline (T=S=1024, E=768, H=12, D=64):
  A^T, G^T via PE transposes -> Q^T,K^T (transposed layout) and V (natural,
  packed with a ones column per head) -> per head: logits[s,t] (K=64 matmul),
  exp on ACT into bf16 u, attention matmul with [V_h|1] lhsT (row 64 gives
  sum_s exp = Z), normalize via reciprocal + K=1 broadcast matmul, running
  max-over-heads of the normalized weights (bf16 DVE) -> output projection
  (fp32r) -> BCE from the max-weight map + target_rel masks.
"""

import numpy as np

T, S, B, E, H = 1024, 1024, 8, 768, 12
D = E // H  # 64
N_CORES = 8
EK = E // 128  # 6 e-tiles
NT = T // 128  # 8 t-tiles
NS = S // 128  # 8 s-tiles

_cache = {}


def _build(stage='full'):
    import concourse.bass as bass  # noqa: F401
    import concourse.bacc as bacc
    import concourse.mybir as mybir
    import concourse.tile as tile

    f32 = mybir.dt.float32
    f32r = mybir.dt.float32r
    bf16 = mybir.dt.bfloat16
    i32 = mybir.dt.int32
    AF = mybir.ActivationFunctionType
    OP = mybir.AluOpType

    nc = bacc.Bacc("TRN2", target_bir_lowering=False, debug=False,
                   num_devices=N_CORES)

    a_d = nc.dram_tensor("a", [T, E], f32, kind="ExternalInput").ap()
    g_d = nc.dram_tensor("g", [S, E], f32, kind="ExternalInput").ap()
    rel_d = nc.dram_tensor("relt", [S, T], i32, kind="ExternalInput").ap()
    wqt_d = nc.dram_tensor("wqt", [E, E], f32r, kind="ExternalInput").ap()
    wkt_d = nc.dram_tensor("wkt", [E, E], f32r, kind="ExternalInput").ap()
    wvt_d = nc.dram_tensor("wvt", [E, E], f32r, kind="ExternalInput").ap()
    wot_d = nc.dram_tensor("wot", [E, E], f32r, kind="ExternalInput").ap()
    bq_d = nc.dram_tensor("bq", [E, 1], f32, kind="ExternalInput").ap()
    bk_d = nc.dram_tensor("bk", [E, 1], f32, kind="ExternalInput").ap()
    bo_d = nc.dram_tensor("bo", [E, 1], f32, kind="ExternalInput").ap()
    id_d = nc.dram_tensor("ident", [128, 128], f32, kind="ExternalInput").ap()
    xt_d = nc.dram_tensor("xt", [E, T], f32, kind="ExternalOutput").ap()
    loss_d = nc.dram_tensor("loss", [1, 1], f32, kind="ExternalOutput").ap()

    with tile.TileContext(nc) as tc:
      with tc.tile_pool(name="sbP", bufs=1) as sbP:
        # ---------------- persistent tiles ----------------
        ident = sbP.tile([128, 128], f32, tag="ident", name="ident")
        nc.sync.dma_start(ident[:], id_d[:])
        ones = sbP.tile([1, 128], bf16, tag="ones", name="ones")
        nc.vector.memset(ones[:], 1.0)
        bq_sb = sbP.tile([128, EK, 1], f32, tag="bq", name="bq_sb")
        nc.sync.dma_start(bq_sb[:], bq_d.rearrange("(k p) o -> p k o", p=128))
        bk_sb = sbP.tile([128, EK, 1], f32, tag="bk", name="bk_sb")
        nc.sync.dma_start(bk_sb[:], bk_d.rearrange("(k p) o -> p k o", p=128))
        bo_sb = sbP.tile([128, EK, 1], f32, tag="bo", name="bo_sb")
        nc.sync.dma_start(bo_sb[:], bo_d.rearrange("(k p) o -> p k o", p=128))
        wot = sbP.tile([128, EK, E], f32r, tag="wot", name="wot")
        nc.sync.dma_start(wot[:], wot_d.rearrange("(k p) n -> p k n", p=128))

        qt = sbP.tile([128, EK, T], bf16, tag="qt", name="qt")
        kt = sbP.tile([128, EK, T], bf16, tag="kt", name="kt")
        vaug = sbP.tile([128, NS, H, D + 1], bf16, tag="vaug", name="vaug")
        m = sbP.tile([128, NS, T], bf16, tag="m", name="m")
        attnT = sbP.tile([128, EK, T], f32r, tag="attnT", name="attnT")

        # ---------------- prologue: transposes + projections ----------------
        with tc.tile_pool(name="sbA", bufs=1) as sbA, \
             tc.tile_pool(name="psA", bufs=1, space="PSUM") as psA:

            def load_T(src_d, dst, nm):
                """dst[e_partition, (ek, token)] = src^T via PE transposes."""
                src_r = src_d.rearrange("(t p) e -> p t e", p=128)
                nat = sbA.tile([128, NT, E], f32, tag="nat", name="nat" + nm)
                for ti in range(NT):
                    nc.sync.dma_start(nat[:, ti, :], src_r[:, ti, :])
                for ek in range(EK):
                    for grp in range(2):
                        tr = psA.tile([128, 512], f32, tag="tr", bufs=2,
                                      name=f"tr{nm}{ek}{grp}")
                        for q in range(4):
                            ti = grp * 4 + q
                            nc.tensor.transpose(
                                tr[:, q * 128:(q + 1) * 128],
                                nat[:, ti, ek * 128:(ek + 1) * 128],
                                ident[:],
                            )
                        nc.scalar.activation(
                            dst[:, ek, grp * 512:(grp + 1) * 512], tr[:],
                            AF.Copy)

            def project(src_t, w_sb, bias_sb, out, nm):
                """out[e', (ek', t)] = W^T.T @ src_t + bias."""
                for j in range(EK):
                    for c in range(2):
                        pp = psA.tile([128, 512], f32, tag="qk", bufs=2,
                                      name=f"pp{nm}{j}{c}")
                        for k in range(EK):
                            nc.tensor.matmul(
                                pp[:],
                                w_sb[:, k, j * 128:(j + 1) * 128],
                                src_t[:, k, c * 512:(c + 1) * 512],
                                start=(k == 0), stop=(k == EK - 1),
                            )
                        nc.scalar.activation(
                            out[:, j, c * 512:(c + 1) * 512], pp[:],
                            AF.Identity, bias_sb[:, j, :])

            wq = sbA.tile([128, EK, E], f32r, tag="wproj", bufs=2, name="wq")
            nc.sync.dma_start(wq[:], wqt_d.rearrange("(k p) n -> p k n", p=128))
            wk = sbA.tile([128, EK, E], f32r, tag="wproj", bufs=2, name="wk")
            nc.sync.dma_start(wk[:], wkt_d.rearrange("(k p) n -> p k n", p=128))

            tact_a = sbA.tile([128, EK, T], f32r, tag="tact", name="tact_a")
            load_T(a_d, tact_a, "a")
            project(tact_a, wq, bq_sb, qt, "q")

            tact_g = sbA.tile([128, EK, T], f32r, tag="tact", name="tact_g")
            load_T(g_d, tact_g, "g")
            project(tact_g, wk, bk_sb, kt, "k")

            wv = sbA.tile([128, EK, E], f32r, tag="wproj", bufs=2, name="wv")
            nc.sync.dma_start(wv[:], wvt_d.rearrange("(k p) n -> p k n", p=128))

            # V natural [s, e''] packed into [V_h | ones] 65-col blocks (bf16)
            nc.vector.memset(vaug[:, :, :, D:D + 1], 1.0)
            for i in range(NS):
                for c2 in range(2):
                    n = 512 if c2 == 0 else 256
                    nh = n // D
                    h0 = c2 * 8
                    vp = psA.tile([128, 8, D], f32, tag="qk", bufs=2,
                                  name=f"vp{i}{c2}")
                    for k in range(EK):
                        nc.tensor.matmul(
                            vp[:, 0:nh, :],
                            tact_g[:, k, i * 128:(i + 1) * 128],
                            wv[:, k, c2 * 512:c2 * 512 + n],
                            start=(k == 0), stop=(k == EK - 1),
                        )
                    nc.scalar.activation(
                        vaug[:, i, h0:h0 + nh, 0:D], vp[:, 0:nh, :], AF.Copy)

        if stage == 'pro':
            with tc.tile_pool(name="sbD", bufs=1) as sbD:
                for j in range(EK):
                    dq = sbD.tile([128, T], f32, tag="dq", bufs=2,
                                  name=f"dq{j}")
                    nc.vector.tensor_copy(dq[:], qt[:, j, :].bitcast(f32))
                    nc.sync.dma_start(
                        xt_d.rearrange("(j p) t -> p j t", p=128)[:, j, :],
                        dq[:])
                ls0 = sbD.tile([1, 1], f32, tag="ls0", name="ls0")
                nc.vector.memset(ls0[:], 0.0)
                nc.sync.dma_start(loss_d[:], ls0[:])

        # ---------------- heads loop ----------------
        if stage not in ('pro',):
          with tc.tile_pool(name="sbH", bufs=1) as sbH, \
             tc.tile_pool(name="psH", bufs=1, space="PSUM") as psH:
            def emit_qk(h):
                pt, po = h // 2, (h % 2) * 64
                u = sbH.tile([128, NS, T], bf16, tag="u", bufs=2,
                             name=f"u{h}")
                for i in range(NS):
                    for c in range(2):
                        lg = psH.tile([128, 512], f32, tag="lg", bufs=2,
                                      name=f"lg{h}{i}{c}")
                        nc.tensor.matmul(
                            lg[:],
                            kt[po:po + 64, pt, i * 128:(i + 1) * 128],
                            qt[po:po + 64, pt, c * 512:(c + 1) * 512],
                            start=True, stop=True,
                        )
                        nc.scalar.activation(
                            u[:, i, c * 512:(c + 1) * 512], lg[:], AF.Exp)
                return u

            def emit_tail(h, u):
                pt, po = h // 2, (h % 2) * 64
                atp = psH.tile([65, 1024], f32, tag="at", bufs=2,
                               name=f"atp{h}")
                for c in range(2):
                    for i in range(NS):
                        nc.tensor.matmul(
                            atp[0:65, c * 512:(c + 1) * 512],
                            vaug[:, i, h, 0:D + 1],
                            u[:, i, c * 512:(c + 1) * 512],
                            start=(i == 0), stop=(i == NS - 1),
                        )
                zrow_bf = sbH.tile([1, T], bf16, tag="zrowbf", bufs=2,
                                   name=f"zrowbf{h}")
                nc.scalar.activation(zrow_bf[:], atp[64:65, :], AF.Copy)
                bc = psH.tile([128, 1024], f32, tag="bc", bufs=1,
                              name=f"bc{h}")
                for c in range(2):
                    nc.tensor.matmul(
                        bc[:, c * 512:(c + 1) * 512],
                        ones[:],
                        zrow_bf[0:1, c * 512:(c + 1) * 512],
                        start=True, stop=True,
                    )
                rz = sbH.tile([128, T], f32, tag="rz", bufs=2, name=f"rz{h}")
                nc.vector.reciprocal_approx_fast(rz[:], bc[:])
                rb = sbH.tile([128, T], bf16, tag="rb", bufs=2, name=f"rb{h}")
                nc.scalar.activation(rb[:], rz[:], AF.Copy)
                # normalized attention output rows for this head
                nc.vector.tensor_tensor(
                    attnT[po:po + 64, pt, :], atp[0:64, :], rb[0:64, :],
                    op=OP.mult)
                # running max over heads of normalized weights
                for i in range(NS):
                    if h == 0:
                        nc.vector.tensor_tensor(
                            m[:, i, :], u[:, i, :], rb[:], op=OP.mult)
                    else:
                        tmp = sbH.tile([128, T], bf16, tag="mtmp", bufs=2,
                                       name=f"tmp{h}{i}")
                        nc.vector.tensor_tensor(
                            tmp[:], u[:, i, :], rb[:], op=OP.mult)
                        nc.vector.tensor_tensor(
                            m[:, i, :], m[:, i, :], tmp[:], op=OP.max)

            for h in range(H):
                u = emit_qk(h)
                emit_tail(h, u)

        # ---------------- tail: output projection + BCE ----------------
        if stage == 'full':
          with tc.tile_pool(name="sbT", bufs=1) as sbT, \
             tc.tile_pool(name="psT", bufs=1, space="PSUM") as psT:
            for j in range(EK):
                for c in range(2):
                    xp = psT.tile([128, 512], f32, tag="xp", bufs=2,
                                  name=f"xp{j}{c}")
                    for k in range(EK):
                        nc.tensor.matmul(
                            xp[:],
                            wot[:, k, j * 128:(j + 1) * 128],
                            attnT[:, k, c * 512:(c + 1) * 512],
                            start=(k == 0), stop=(k == EK - 1),
                        )
                    xo = sbT.tile([128, 512], f32, tag="xo", bufs=3,
                                  name=f"xo{j}{c}")
                    nc.scalar.activation(xo[:], xp[:], AF.Identity,
                                         bo_sb[:, j, :])
                    nc.sync.dma_start(
                        xt_d.rearrange("(j p) t -> p j t",
                                       p=128)[:, j, c * 512:(c + 1) * 512],
                        xo[:])

            relt = sbT.tile([128, NS, T], i32, tag="relt", name="relt")
            rel_r = rel_d.rearrange("(i p) t -> p i t", p=128)
            acc = sbT.tile([128, 2 * NS], f32, tag="acc", name="acc")
            for i in range(NS):
                nc.sync.dma_start(relt[:, i, :], rel_r[:, i, :])
                mlog = sbT.tile([128, T], f32, tag="mlog", bufs=2,
                                name=f"mlog{i}")
                nc.scalar.activation(mlog[:], m[:, i, :], AF.Ln)
                nc.vector.tensor_scalar_max(mlog[:], mlog[:], -100.0)
                l1m = sbT.tile([128, T], f32, tag="l1m", bufs=2,
                               name=f"l1m{i}")
                nc.scalar.activation(l1m[:], m[:, i, :], AF.Ln, 1.0, -1.0)
                nc.vector.tensor_scalar_max(l1m[:], l1m[:], -100.0)
                af = sbT.tile([128, T], f32, tag="af", bufs=2, name=f"af{i}")
                nc.vector.tensor_scalar(af[:], relt[:, i, :], 1, None,
                                        op0=OP.is_equal)
                bf_ = sbT.tile([128, T], f32, tag="bf", bufs=2, name=f"bf{i}")
                nc.vector.tensor_scalar(bf_[:], relt[:, i, :], 2, None,
                                        op0=OP.is_equal)
                dump = sbT.tile([128, T], f32, tag="dump", bufs=2,
                                name=f"dump{i}")
                nc.vector.tensor_tensor_reduce(
                    out=dump[:], in0=af[:], in1=mlog[:], scale=-1.0,
                    scalar=0.0, op0=OP.mult, op1=OP.add,
                    accum_out=acc[:, 2 * i:2 * i + 1])
                nc.vector.tensor_tensor_reduce(
                    out=dump[:], in0=bf_[:], in1=l1m[:], scale=-1.0,
                    scalar=0.0, op0=OP.mult, op1=OP.add,
                    accum_out=acc[:, 2 * i + 1:2 * i + 2])
            import concourse.bass_isa as bass_isa
            accv = sbT.tile([128, 1], f32, tag="accv", name="accv")
            nc.vector.reduce_sum(accv[:], acc[:], axis=mybir.AxisListType.X)
            accr = sbT.tile([128, 1], f32, tag="accr", name="accr")
            nc.gpsimd.partition_all_reduce(accr[:], accv[:], 128,
                                           bass_isa.ReduceOp.add)
            nc.sync.dma_start(loss_d[:], accr[0:1, 0:1])

    nc.compile()
    return nc


def _numpy_fallback(outs, graph_state, graph_padding_mask, attn_mask,
                    strategy_id, target_rel, Win, b_in, Wout, b_out):
    scaling = D ** -0.5
    q = (outs @ Win[:E].T + b_in[:E]) * scaling
    k = graph_state @ Win[E:2 * E].T + b_in[E:2 * E]
    v = graph_state @ Win[2 * E:].T + b_in[2 * E:]
    q = q.reshape(T, B, H, D)
    k = k.reshape(S, B, H, D)
    v = v.reshape(S, B, H, D)
    logits = np.einsum('tbhd,sbhd->bhts', q, k)
    logits = logits + attn_mask[None, None]
    logits = np.where(graph_padding_mask[:, None, None, :], -np.inf, logits)
    logits -= logits.max(-1, keepdims=True)
    w = np.exp(logits)
    w /= w.sum(-1, keepdims=True)
    arc_weight = np.transpose(w.max(1), (1, 0, 2))
    attn = np.einsum('bhts,sbhd->tbhd', w, v).reshape(T, B, E)
    x = attn @ Wout.T + b_out
    target_arc = (target_rel != 2).astype(np.float32)
    with np.errstate(divide='ignore'):
        logp = np.maximum(np.log(arc_weight), -100.0)
        log1mp = np.maximum(np.log1p(-arc_weight), -100.0)
    bce = -(target_arc * logp + (1.0 - target_arc) * log1mp)
    bce = np.where(target_rel == 0, 0.0, bce)
    arc_loss = bce.sum((0, 2)) * strategy_id
    return arc_loss.astype(np.float32), x.astype(np.float32)


def kernel(ids, step, outs, graph_state, graph_padding_mask, attn_mask,
           strategy_id, target_rel, Win, b_in, Wout, b_out):
    outs_in = outs
    outs = np.asarray(outs, np.float32)
    graph_state = np.asarray(graph_state, np.float32)
    graph_padding_mask = np.asarray(graph_padding_mask)
    attn_mask = np.asarray(attn_mask, np.float32)
    strategy_id = np.asarray(strategy_id, np.float32)
    target_rel_np = np.asarray(target_rel)
    Win = np.asarray(Win, np.float32)
    b_in = np.asarray(b_in, np.float32)
    Wout = np.asarray(Wout, np.float32)
    b_out = np.asarray(b_out, np.float32)

    if attn_mask.any() or graph_padding_mask.any():
        arc_loss, x = _numpy_fallback(outs, graph_state, graph_padding_mask,
                                      attn_mask, strategy_id, target_rel_np,
                                      Win, b_in, Wout, b_out)
        return arc_loss, outs_in, x

    from concourse.bass_utils import run_bass_kernel_spmd

    scaling = np.float32(D ** -0.5)
    WqT = np.ascontiguousarray((Win[:E] * scaling).T)
    WkT = np.ascontiguousarray(Win[E:2 * E].T)
    WvT = np.ascontiguousarray(Win[2 * E:].T)
    WoT = np.ascontiguousarray(Wout.T)
    bq = (b_in[:E] * scaling).reshape(E, 1).copy()
    bk = b_in[E:2 * E].reshape(E, 1).copy()
    bo = (b_out + Wout @ b_in[2 * E:]).reshape(E, 1).copy()
    ident = np.eye(128, dtype=np.float32)

    in_maps = []
    for b in range(B):
        in_maps.append({
            "a": np.ascontiguousarray(outs[:, b, :]),
            "g": np.ascontiguousarray(graph_state[:, b, :]),
            "relt": np.ascontiguousarray(
                target_rel_np[:, b, :].T).astype(np.int32),
            "wqt": WqT, "wkt": WkT, "wvt": WvT, "wot": WoT,
            "bq": bq, "bk": bk, "bo": bo,
            "ident": ident,
        })

    if "nc" not in _cache:
        _cache["nc"] = _build()
    nc = _cache["nc"]

    res = run_bass_kernel_spmd(nc, in_maps, core_ids=list(range(N_CORES)))

    x = np.empty((T, B, E), np.float32)
    arc_loss = np.empty((B,), np.float32)
    for b in range(B):
        x[:, b, :] = res.results[b]["xt"].T
        arc_loss[b] = res.results[b]["loss"][0, 0] * strategy_id[b]
    return arc_loss, outs_in, x
